# revision 1
# baseline (speedup 1.0000x reference)
"""Segment-max (MIL pooling) Bass kernel for Trainium2, 8 NeuronCores.

Strategy:
  - Host computes bag offsets from bags_size (cumsum) and shards bags across
    8 cores as contiguous ranges balanced by padded row count.
  - Each bag is padded with -inf rows up to a small set of "class" sizes
    (multiples of G=8 rows) when the per-core slab is assembled, so bag
    starts are G-aligned and gathers never read a neighbouring bag.
  - Device kernel (SPMD, identical program on all 8 cores): for each slot
    (one class-group of up to 128 bags) a SWDGE dma_gather pulls 128 padded
    bags from HBM, one bag per SBUF partition; a single strided
    vector.reduce_max then reduces all 128 bags' rows at once.
  - Output tiles are written back once; host scatters rows back to the
    original bag order.
"""

import os
import sys
import types
import tempfile
from contextlib import ExitStack

import numpy as np

K = 64          # feature dim
G = 8           # row alignment granule; elem_step = G*K elements = 2048 B
P = 128
N_CORES = 8
NEG = np.float32(-np.inf)

LAST = {}       # exec_time_ns etc. from the most recent run (for test.py)


def _install_hooks():
    """Restore the NTFF profiling hook under axon + disable artifact upload."""
    try:
        if "antenv.axon_hooks" not in sys.modules:
            mod = types.ModuleType("antenv.axon_hooks")
            mod._hook = None

            def set_h(h):
                mod._hook = h

            def get_h():
                return mod._hook

            mod.set_axon_ntff_profile_hook = set_h
            mod.get_axon_ntff_profile_hook = get_h
            sys.modules["antenv.axon_hooks"] = mod
            from trn_agent_boot.trn_boot import _ntff_profile_via_ctypes

            hook = _ntff_profile_via_ctypes("/opt/axon/libaxon_pjrt.so")
            mod.set_axon_ntff_profile_hook(hook)
    except Exception:
        pass
    try:
        import concourse.bass_utils as bu

        bu.upload_artifacts = lambda tmpdir: str(tmpdir)
    except Exception:
        pass


def _build(slot_sizes, n_slots, r_pad, nbuf=4):
    import concourse.bass as bass
    import concourse.bacc as bacc
    import concourse.mybir as mybir

    icols = (P + 15) // 16  # idx columns per slot (128 idx wrapped in 16 parts)
    smax = max(slot_sizes)
    nc = bacc.Bacc("TRN2", target_bir_lowering=False, debug=False,
                   num_devices=N_CORES)
    slab = nc.dram_tensor("slab", [r_pad, K], mybir.dt.float32,
                          kind="ExternalInput")
    idxs = nc.dram_tensor("idxs", [P, icols * n_slots], mybir.dt.int16,
                          kind="ExternalInput")
    out = nc.dram_tensor("out", [P, n_slots * K], mybir.dt.float32,
                         kind="ExternalOutput")

    with ExitStack() as stack:
        idx_sb = stack.enter_context(
            nc.sbuf_tensor("idx_sb", [P, icols * n_slots], mybir.dt.int16))
        bufs = stack.enter_context(
            nc.sbuf_tensor("bufs", [P, nbuf * smax * K], mybir.dt.float32))
        outb = stack.enter_context(
            nc.sbuf_tensor("outb", [P, n_slots * K], mybir.dt.float32))
        idx_sem = stack.enter_context(nc.semaphore("idx_sem"))
        gsem = [stack.enter_context(nc.semaphore(f"g{i}")) for i in range(nbuf)]
        rsem = stack.enter_context(nc.semaphore("red"))
        osem = stack.enter_context(nc.semaphore("outd"))
        block = stack.enter_context(nc.Block())

        @block.gpsimd
        def _(gp):
            gp.dma_start(idx_sb[:], idxs.ap()).then_inc(idx_sem, 16)
            gp.wait_ge(idx_sem, 16)
            for j, S in enumerate(slot_sizes):
                if j >= nbuf:
                    gp.wait_ge(rsem, j - nbuf + 1)
                b = j % nbuf
                dst = bufs[:, b * smax * K: b * smax * K + S * K].rearrange(
                    "p (j e) -> p j e", j=1)
                nblk = (r_pad - S) // G + 1
                src = bass.AP(slab.ap().tensor, 0, [[G * K, nblk], [1, S * K]])
                gp.dma_gather(dst, src,
                              idx_sb[:, j * icols:(j + 1) * icols],
                              P, P, S * K, elem_step=G * K
                              ).then_inc(gsem[b], 16)

        @block.vector
        def _(v):
            for j, S in enumerate(slot_sizes):
                b = j % nbuf
                v.wait_ge(gsem[b], 16 * (j // nbuf + 1))
                view = bufs[:, b * smax * K: b * smax * K + S * K].rearrange(
                    "p (r k) -> p k r", k=K)
                v.reduce_max(outb[:, j * K:(j + 1) * K], view,
                             axis=mybir.AxisListType.X).then_inc(rsem, 1)

        @block.sync
        def _(sp):
            sp.wait_ge(rsem, n_slots)
            sp.dma_start(out.ap(), outb[:]).then_inc(osem, 16)
            sp.wait_ge(osem, 16)

    nc.compile()
    return nc


def kernel(inter_pre, bags_size):
    _install_hooks()
    from concourse.bass_utils import run_bass_kernel_spmd

    X = np.ascontiguousarray(np.asarray(inter_pre, dtype=np.float32))
    sizes = np.asarray(bags_size, dtype=np.int64)
    n_bags = int(sizes.shape[0])
    assert X.shape[1] == K
    starts = np.zeros(n_bags, np.int64)
    np.cumsum(sizes[:-1], out=starts[1:])

    # ---- classes: G-aligned pad sizes, merged upward until global count>=8P
    pad0 = ((sizes + G - 1) // G) * G
    uniq, counts = np.unique(pad0, return_counts=True)
    classes = []
    acc = 0
    for i, (u, cnt) in enumerate(zip(uniq, counts)):
        acc += int(cnt)
        if acc >= 8 * P or i == len(uniq) - 1:
            classes.append(int(u))
            acc = 0
    classes = np.array(classes, np.int64)
    cls_of = np.searchsorted(classes, pad0, side="left")
    padded = classes[cls_of]

    # ---- shard: contiguous ranges balanced by padded rows
    cum = np.cumsum(padded)
    tgt = cum[-1] / N_CORES
    bounds = [0]
    for c in range(1, N_CORES):
        bounds.append(int(np.searchsorted(cum, tgt * c)))
    bounds.append(n_bags)

    # ---- global slot structure: per class, max over cores of ceil(cnt/P)
    ncls = len(classes)
    core_cls_cnt = np.zeros((N_CORES, ncls), np.int64)
    for c in range(N_CORES):
        cc = cls_of[bounds[c]:bounds[c + 1]]
        core_cls_cnt[c] = np.bincount(cc, minlength=ncls)
    slots_per_cls = np.maximum(1, -(-core_cls_cnt.max(axis=0) // P))
    slot_sizes = []
    slot_cls = []
    for k in range(ncls):
        for _ in range(int(slots_per_cls[k])):
            slot_sizes.append(int(classes[k]))
            slot_cls.append(k)
    n_slots = len(slot_sizes)

    # ---- per-core slab row budget (uniform R_PAD)
    core_rows = np.array([padded[bounds[c]:bounds[c + 1]].sum()
                          for c in range(N_CORES)])
    r_pad = int(core_rows.max())
    r_pad = max(r_pad, max(slot_sizes))
    r_pad = ((r_pad + G - 1) // G) * G

    icols = (P + 15) // 16
    slabs, idx_arrays = [], []
    bag_map = np.full((N_CORES, n_slots, P), -1, np.int64)
    for c in range(N_CORES):
        b0, b1 = bounds[c], bounds[c + 1]
        sz_c = sizes[b0:b1]
        pad_c = padded[b0:b1]
        dst_starts = np.zeros(b1 - b0, np.int64)
        np.cumsum(pad_c[:-1], out=dst_starts[1:])
        # vectorized slab build
        slab = np.full((r_pad, K), NEG, np.float32)
        nrows = int(sz_c.sum())
        rel = np.arange(nrows, dtype=np.int64)
        src_rel_start = np.zeros(b1 - b0, np.int64)
        np.cumsum(sz_c[:-1], out=src_rel_start[1:])
        dst_rows = np.repeat(dst_starts, sz_c) + (rel - np.repeat(src_rel_start, sz_c))
        slab[dst_rows] = X[starts[b0]:starts[b0] + nrows]
        slabs.append(slab)

        # idx assignment: bags of class k fill that class's slots in order
        idx_vals = np.zeros((n_slots, P), np.int64)
        cls_c = cls_of[b0:b1]
        slot_base = {}
        si = 0
        for k in range(ncls):
            slot_base[k] = si
            si += int(slots_per_cls[k])
        for k in range(ncls):
            sel = np.nonzero(cls_c == k)[0]
            base = slot_base[k]
            nsl = int(slots_per_cls[k])
            # fill slots with real bags
            for t in range(nsl):
                lanes = sel[t * P:(t + 1) * P]
                j = base + t
                nl = len(lanes)
                if nl > 0:
                    idx_vals[j, :nl] = dst_starts[lanes] // G
                    bag_map[c, j, :nl] = lanes + b0
                # dup-pad remaining lanes with first lane's idx (or block 0)
                fill = idx_vals[j, 0] if nl > 0 else 0
                idx_vals[j, nl:] = fill
        assert idx_vals.max() <= (r_pad - max(slot_sizes)) // G, \
            (idx_vals.max(), r_pad)
        # wrap: idx i -> partition i%16, column i//16; tile to 128 partitions
        arr = np.zeros((P, icols * n_slots), np.int16)
        for j in range(n_slots):
            w = idx_vals[j].reshape(icols, 16).T.astype(np.int16)  # [16, icols]
            arr[:, j * icols:(j + 1) * icols] = np.tile(w, (8, 1))
        idx_arrays.append(arr)

    nc = _build(slot_sizes, n_slots, r_pad)
    in_maps = [{"slab": slabs[c], "idxs": idx_arrays[c]}
               for c in range(N_CORES)]
    trace = bool(os.environ.get("SEG_TRACE"))
    tmpdir = os.environ.get("SEG_TRACE_DIR") or tempfile.mkdtemp()
    res = run_bass_kernel_spmd(nc, in_maps, core_ids=list(range(N_CORES)),
                               trace=trace, tmpdir=tmpdir)
    LAST["exec_time_ns"] = res.exec_time_ns
    LAST["mean_exec_time_ns"] = res.mean_exec_time_ns
    LAST["tmpdir"] = tmpdir

    outp = np.empty((n_bags, K), np.float32)
    seen = np.zeros(n_bags, bool)
    for c in range(N_CORES):
        o = res.results[c]["out"].reshape(P, n_slots, K)
        jj, pp = np.nonzero(bag_map[c].T >= 0)  # careful: bag_map is [j, p]
        # bag_map[c] shape [n_slots, P]; iterate valid entries
        valid_j, valid_p = np.nonzero(bag_map[c] >= 0)
        ids = bag_map[c, valid_j, valid_p]
        outp[ids] = o[valid_p, valid_j]
        seen[ids] = True
    assert seen.all()
    return outp
